# revision 20
# baseline (speedup 1.0000x reference)
"""Causal multi-head attention (B=2, S=2048, D=1024, H=16) on 8 trn2
NeuronCores.

Sharding (per the head-parallel hint): core c handles batch c//4 and heads
4*(c%4) .. 4*(c%4)+3 (a 256-wide slice of the q/k/v feature dim).  W_proj is
tensor-parallel split along the head dim, so each core emits a full-shape
[S, D] partial projection output; the host sums the 4 partials per batch.

v3 strategy (all-bf16 data path, fp32 PSUM accumulation):
  - host feeds x[b].T in bf16 so the contraction dim (d) lands on partitions
  - q/k pair-0 projection runs per-contraction-chunk into 8 persistent PSUM
    banks so the PE starts on the first x chunk; input DMAs are split and
    ordered by first use
  - qT/kT in transposed [dh, s] bf16 layout (2 head-pairs of 128); v in
    natural [s, dh] layout interleaved per head as [64 v | 64 ones] so the
    AV matmul emits replicated softmax denominators
  - the attention inner loop is software-pipelined: AV(j-1) is emitted after
    scores(j), with "filler" matmuls (pair-1 q/k projection during pair-0
    attention, output projection of the previous quarter during pair-1
    attention) interleaved between strips so the PE never idles while the
    scalar engine runs Exp -- attention is Exp-bound on ACT otherwise
  - diagonal blocks masked by accumulating a bf16 matmul (strict-upper -1000
    against identity) before Exp; no max-subtraction needed
"""

import os

import numpy as np

# cache compiled executables (incl. the wrapped NEFF) across processes
os.environ.setdefault("JAX_COMPILATION_CACHE_DIR", "/tmp/jax_comp_cache")
os.environ.setdefault("JAX_PERSISTENT_CACHE_MIN_ENTRY_SIZE_BYTES", "0")
os.environ.setdefault("JAX_PERSISTENT_CACHE_MIN_COMPILE_TIME_SECS", "0")

S = 2048
D = 1024
DH = 64
P = 128
NT = S // P   # 16 sequence tiles
DC = D // P   # 8 contraction chunks
MASK_C = 1000.0
N_CORES = 8

_CACHE = {}


def _build_bass():
    from collections import deque

    import concourse.bass as bass
    import concourse.tile as tile
    from concourse import mybir

    f32 = mybir.dt.float32
    bf16 = mybir.dt.bfloat16
    EXP = mybir.ActivationFunctionType.Exp

    nc = bass.Bass("TRN2")

    xT_d = nc.dram_tensor("xT", [D, S], bf16, kind="ExternalInput")
    wq_d = nc.dram_tensor("wq_t", [D, 256], bf16, kind="ExternalInput")
    wk_d = nc.dram_tensor("wk_t", [D, 256], bf16, kind="ExternalInput")
    wv_d = nc.dram_tensor("wv_t", [D, 256], bf16, kind="ExternalInput")
    wp_d = nc.dram_tensor("wp_t", [256, D], bf16, kind="ExternalInput")
    mask_d = nc.dram_tensor("mask_lhsT", [P, P], bf16, kind="ExternalInput")
    ident_d = nc.dram_tensor("ident", [P, P], bf16, kind="ExternalInput")
    out_d = nc.dram_tensor("out", [S, D], bf16, kind="ExternalOutput")

    with tile.TileContext(nc) as tc:
        with tc.tile_pool(name="persist", bufs=1) as persist:
            qT = [persist.tile([P, S], bf16, name=f"qT{p}", tag=f"qT{p}")
                  for p in range(2)]
            kT = [persist.tile([P, S], bf16, name=f"kT{p}", tag=f"kT{p}")
                  for p in range(2)]
            v4e = [persist.tile([P, 4, P], bf16, name=f"v4e{t}", tag=f"v4e{t}")
                   for t in range(NT)]
            wp_sb = persist.tile([P, 2, D], bf16, name="wp", tag="wp")
            attnT = [persist.tile([P, S], bf16, name=f"attnT{p}",
                                  tag=f"attnT{p}") for p in range(2)]
            mask_sb = persist.tile([P, P], bf16, name="mask_sb", tag="mask_sb")
            ident_sb = persist.tile([P, P], bf16, name="ident_sb",
                                    tag="ident_sb")

            with tc.tile_pool(name="xw", bufs=1) as xw:
                xsb = xw.tile([P, DC, S], bf16, name="xsb", tag="xsb")
                wq_sb = xw.tile([P, DC, 256], bf16, name="wq", tag="wq")
                wk_sb = xw.tile([P, DC, 256], bf16, name="wk", tag="wk")
                wv_sb = xw.tile([P, DC, 256], bf16, name="wv", tag="wv")

                # DMA order = first-use order on the PE (split for latency)
                wqv = wq_d[:].rearrange("(c p) n -> p c n", p=P)
                wkv = wk_d[:].rearrange("(c p) n -> p c n", p=P)
                xv = xT_d[:].rearrange("(c p) s -> p c s", p=P)
                nc.sync.dma_start(out=wq_sb[:, 0:2, :], in_=wqv[:, 0:2, :])
                nc.sync.dma_start(out=xsb[:, 0, 0:1024], in_=xv[:, 0, 0:1024])
                nc.sync.dma_start(out=xsb[:, 0, 1024:S], in_=xv[:, 0, 1024:S])
                nc.sync.dma_start(out=wq_sb[:, 2:8, :], in_=wqv[:, 2:8, :])
                nc.sync.dma_start(out=xsb[:, 1:2, :], in_=xv[:, 1:2, :])
                for i in range(1, 4):
                    nc.sync.dma_start(
                        out=xsb[:, 2 * i:2 * i + 2, :],
                        in_=xv[:, 2 * i:2 * i + 2, :])
                nc.sync.dma_start(out=wk_sb[:], in_=wkv[:])
                nc.sync.dma_start(
                    out=wv_sb[:],
                    in_=wv_d[:].rearrange("(c p) n -> p c n", p=P))
                nc.sync.dma_start(out=mask_sb[:], in_=mask_d[:])
                nc.sync.dma_start(out=ident_sb[:], in_=ident_d[:])
                nc.sync.dma_start(
                    out=wp_sb[:],
                    in_=wp_d[:].rearrange("(c p) n -> p c n", p=P))

                # ones halves of v4e (constant)
                for t in range(NT):
                    nc.vector.memset(v4e[t][:, :, 64:P], 1.0)

                # ---- phase 1a: q for BOTH pairs, chunk-major into all 8
                # PSUM banks (1.7us of PE work per 1.46us x-chunk transfer,
                # so the PE tracks the DMA stream without starving) ----
                with tc.tile_pool(name="pk0", bufs=1, space="PSUM") as pk0:
                    qg = [pk0.tile([P, 512], f32, name=f"qg{p}{n}",
                                   tag=f"qg{p}{n}")
                          for p in range(2) for n in range(4)]
                    for c in range(DC):
                        for g, (p, n) in enumerate(
                                (p, n) for p in range(2) for n in range(4)):
                            nc.tensor.matmul(
                                qg[g][:],
                                lhsT=wq_sb[:, c, p * P:(p + 1) * P],
                                rhs=xsb[:, c, n * 512:(n + 1) * 512],
                                start=(c == 0), stop=(c == DC - 1),
                            )
                            if c == DC - 1:
                                nc.vector.tensor_copy(
                                    qT[p][:, n * 512:(n + 1) * 512], qg[g][:])

                    # ---- phase 1b: k pair 0 tile-major (reusing the q
                    # banks, per-tile deps on single copies), then v tiles
                    # 0..3 eagerly (needed by the first attention quarter)
                    for n in range(4):
                        psh = qg[n][:]
                        for c in range(DC):
                            nc.tensor.matmul(
                                psh,
                                lhsT=wk_sb[:, c, 0:P],
                                rhs=xsb[:, c, n * 512:(n + 1) * 512],
                                start=(c == 0), stop=(c == DC - 1),
                            )
                        nc.vector.tensor_copy(
                            kT[0][:, n * 512:(n + 1) * 512], psh)
                    for t in range(4):
                        psv = qg[4 + t][:, 0:256]
                        for c in range(DC):
                            nc.tensor.matmul(
                                psv,
                                lhsT=xsb[:, c, t * P:(t + 1) * P],
                                rhs=wv_sb[:, c, :],
                                start=(c == 0), stop=(c == DC - 1),
                            )
                        nc.vector.tensor_copy(
                            v4e[t][:, :, 0:64],
                            psv.rearrange("p (h d) -> p h d", h=4))

                # ------- phase 2+3: attention with interleaved fillers -------
                with tc.tile_pool(name="att", bufs=4) as att, \
                     tc.tile_pool(name="rc", bufs=4) as rc, \
                     tc.tile_pool(name="po", bufs=3) as po, \
                     tc.tile_pool(name="ps_s", bufs=2, space="PSUM") as ps_s, \
                     tc.tile_pool(name="ps_a", bufs=2, space="PSUM") as ps_a, \
                     tc.tile_pool(name="fil", bufs=2, space="PSUM") as fil:

                    v_queue = deque()
                    k1_queue = deque()
                    proj_queue = deque()
                    v_done = [4]   # tiles 0..3 built eagerly above
                    k1_done = [0]

                    def v_units():
                        # v tiles 4..15 as single-matmul filler units
                        for t in range(4, NT):
                            box = {}

                            def mkv(c, t=t, box=box):
                                def f():
                                    if "t" not in box:
                                        box["t"] = fil.tile(
                                            [P, 512], f32,
                                            name=f"vf{t}", tag="fil")
                                    nc.tensor.matmul(
                                        box["t"][:, 0:256],
                                        lhsT=xsb[:, c, t * P:(t + 1) * P],
                                        rhs=wv_sb[:, c, :],
                                        start=(c == 0), stop=(c == DC - 1),
                                    )
                                    if c == DC - 1:
                                        nc.vector.tensor_copy(
                                            v4e[t][:, :, 0:64],
                                            box["t"][:, 0:256].rearrange(
                                                "p (h d) -> p h d", h=4))
                                        v_done[0] += 1
                                return f

                            for c in range(DC):
                                yield mkv(c)

                    def k1_units():
                        # k pair-1 projection as filler units, n-major so
                        # pair-1 attention quarter qc needs groups n <= qc
                        for n in range(4):
                            box = {}

                            def mkk(c, n=n, box=box):
                                def f():
                                    if "t" not in box:
                                        box["t"] = fil.tile(
                                            [P, 512], f32,
                                            name=f"k1_{n}", tag="fil")
                                    nc.tensor.matmul(
                                        box["t"][:],
                                        lhsT=wk_sb[:, c, P:2 * P],
                                        rhs=xsb[:, c,
                                                n * 512:(n + 1) * 512],
                                        start=(c == 0), stop=(c == DC - 1),
                                    )
                                    if c == DC - 1:
                                        nc.vector.tensor_copy(
                                            kT[1][:, n * 512:(n + 1) * 512],
                                            box["t"][:])
                                        k1_done[0] += 1
                                return f

                            for c in range(DC):
                                yield mkk(c)

                    def proj_units(qc):
                        # output projection of quarter qc as filler units
                        for ti in range(4):
                            t = 4 * qc + ti
                            box = {}

                            def mkp(oc, p, t=t, ti=ti, box=box):
                                def f():
                                    if oc == 0 and p == 0:
                                        box["ot"] = po.tile(
                                            [P, D], bf16, name="ot", tag="ot")
                                    if p == 0:
                                        box["ps"] = fil.tile(
                                            [P, 512], f32, name="pso",
                                            tag="fil")
                                    nc.tensor.matmul(
                                        box["ps"][:],
                                        lhsT=attnT[p][:, t * P:(t + 1) * P],
                                        rhs=wp_sb[:, p,
                                                  oc * 512:(oc + 1) * 512],
                                        start=(p == 0), stop=(p == 1),
                                    )
                                    if p == 1:
                                        if qc == 3 and (ti + oc) % 2 == 0:
                                            nc.scalar.copy(
                                                box["ot"][:, oc * 512:
                                                          (oc + 1) * 512],
                                                box["ps"][:])
                                        else:
                                            nc.vector.tensor_copy(
                                                box["ot"][:, oc * 512:
                                                          (oc + 1) * 512],
                                                box["ps"][:])
                                    if p == 1 and oc == 1:
                                        nc.sync.dma_start(
                                            out=out_d[t * P:(t + 1) * P, :],
                                            in_=box["ot"][:])
                                return f

                            for oc in range(2):
                                for p in range(2):
                                    yield mkp(oc, p)

                    def take(n):
                        for _ in range(n):
                            if v_queue:
                                v_queue.popleft()()
                            elif k1_queue:
                                k1_queue.popleft()()
                            elif proj_queue:
                                proj_queue.popleft()()
                            else:
                                return

                    def attention(pr, qc, per_strip):
                        qh = [qT[pr][h * 64:(h + 1) * 64, :] for h in range(2)]
                        kh = [kT[pr][h * 64:(h + 1) * 64, :] for h in range(2)]
                        c0 = qc * 512
                        pa = [ps_a.tile([P, 512], f32, name=f"pa{pr}{qc}{h}",
                                        tag=f"pa{h}", bufs=1)
                              for h in range(2)]
                        jmax = 4 * qc + 3
                        pend = None  # deferred AV: (j, lo, w, et)
                        for j in range(jmax + 1):
                            w0 = j * P
                            lo = max(w0, c0)
                            w = c0 + 512 - lo
                            strip = ps_s.tile([P, 1024], f32,
                                              name="strip", tag="strip")
                            for h in range(2):
                                nc.tensor.matmul(
                                    strip[:, h * 512 + lo - c0:
                                          h * 512 + lo - c0 + w],
                                    lhsT=kh[h][:, w0:w0 + P],
                                    rhs=qh[h][:, lo:lo + w],
                                    start=True, stop=(j // 4 != qc),
                                    skip_group_check=True,
                                )
                            if j // 4 == qc:
                                for h in range(2):
                                    nc.tensor.matmul(
                                        strip[:, h * 512 + w0 - c0:
                                              h * 512 + w0 - c0 + P],
                                        lhsT=mask_sb[:],
                                        rhs=ident_sb[:],
                                        start=False, stop=True,
                                        skip_group_check=True,
                                    )
                            et = att.tile([P, 1024], bf16, name="et", tag="et")
                            sv = strip.rearrange("p (h q) -> p h q", h=2)
                            ev = et.rearrange("p (h q) -> p h q", h=2)
                            nc.scalar.activation(
                                out=ev[:, :, lo - c0:lo - c0 + w],
                                in_=sv[:, :, lo - c0:lo - c0 + w],
                                func=EXP)
                            take(per_strip)
                            if pend is not None:
                                _av(pr, pa, c0, jmax, *pend)
                            pend = (j, lo, w, et)
                        take(per_strip)
                        _av(pr, pa, c0, jmax, *pend)

                    def _av(pr, pa, c0, jmax, j, lo, w, et):
                        for h in range(2):
                            nc.tensor.matmul(
                                pa[h][:, lo - c0:lo - c0 + w],
                                lhsT=v4e[j][:, 2 * pr + h, :],
                                rhs=et[:, h * 512 + lo - c0:
                                       h * 512 + lo - c0 + w],
                                start=(j == 0), stop=(j == jmax),
                                skip_group_check=True,
                            )
                            if j == jmax:
                                # normalize this head immediately: the DVE
                                # chain overlaps the other head's AV
                                recip = rc.tile([64, 512], f32, name="recip",
                                                tag="recip")
                                nc.vector.reciprocal(recip[:], pa[h][64:P, :])
                                nc.vector.tensor_mul(
                                    attnT[pr][h * 64:(h + 1) * 64,
                                              c0:c0 + 512],
                                    pa[h][0:64, :],
                                    recip[:],
                                )

                    v_queue.extend(v_units())
                    k1_queue.extend(k1_units())
                    for qc in range(4):
                        # quarter qc's AV consumes v tiles up to 4*qc+3
                        while v_done[0] < min(4 * qc + 4, NT) and v_queue:
                            v_queue.popleft()()
                        attention(0, qc, per_strip=2)
                    for qc in range(4):
                        # pair-1 quarter qc reads kT[1] cols < 512*(qc+1)
                        while v_queue:
                            v_queue.popleft()()
                        while k1_done[0] < qc + 1 and k1_queue:
                            k1_queue.popleft()()
                        attention(1, qc, per_strip=2)
                        proj_queue.extend(proj_units(qc))
                    while proj_queue:
                        proj_queue.popleft()()

    return nc


def _fix_matmul_waits(nc):
    """The TRN2 ISA events struct holds exactly ONE sync-wait per
    instruction and walrus codegen refuses instructions carrying more
    ("Too many sync wait commands").  Tile emits multi-wait instructions,
    so legalize: hoist excess waits onto single-wait NoOps inserted right
    before the instruction on the same engine -- engine FIFO order
    preserves the synchronization semantics."""
    import bass_rust
    import concourse.mybir as mybir

    n = 0
    for bb in nc.main_func.blocks:
        insts = bb.instructions
        i = 0
        while i < len(insts):
            ins = insts[i]
            si = getattr(ins, "sync_info", None)
            if si is not None and len(si.on_wait) >= 2:
                for w in si.on_wait[:-1]:
                    nop = mybir.InstNoOp(name=f"I-xwait-{n}", ins=[], outs=[])
                    nop.engine = ins.engine
                    nop.sync_info = bass_rust.SyncInfo(
                        on_wait=[w], on_update=[])
                    insts.insert(i, nop)
                    n += 1
                    i += 1
                ins.sync_info = bass_rust.SyncInfo(
                    on_wait=[si.on_wait[-1]], on_update=si.on_update)
            i += 1
    return n


def get_nc(legalize=True):
    key = ("nc", legalize)
    if key not in _CACHE:
        nc = _build_bass()
        if legalize:
            _fix_matmul_waits(nc)
        _CACHE[key] = nc
    return _CACHE[key]


def make_in_maps(x, W_q, W_k, W_v, W_proj):
    import ml_dtypes

    bf = ml_dtypes.bfloat16
    x = np.asarray(x, np.float32)
    W_q = np.asarray(W_q, np.float32)
    W_k = np.asarray(W_k, np.float32)
    W_v = np.asarray(W_v, np.float32)
    W_proj = np.asarray(W_proj, np.float32)

    mask = np.triu(np.full((P, P), -MASK_C, np.float32), k=1).astype(bf)
    ident = np.eye(P, dtype=bf)

    xTs = [np.ascontiguousarray(x[b].T).astype(bf) for b in range(2)]
    in_maps = []
    for core in range(N_CORES):
        b = core // 4
        g = core % 4
        rs = slice(g * 256, (g + 1) * 256)
        in_maps.append({
            "xT": xTs[b],
            "wq_t": (np.ascontiguousarray(W_q[rs].T) / 8.0).astype(bf),
            "wk_t": np.ascontiguousarray(W_k[rs].T).astype(bf),
            "wv_t": np.ascontiguousarray(W_v[rs].T).astype(bf),
            "wp_t": np.ascontiguousarray(W_proj[:, rs].T).astype(bf),
            "mask_lhsT": mask,
            "ident": ident,
        })
    return in_maps


def kernel(x, W_q, W_k, W_v, W_proj, _results_hook=None):
    from concourse.bass_utils import run_bass_kernel_spmd

    nc = get_nc()
    in_maps = make_in_maps(x, W_q, W_k, W_v, W_proj)
    res = run_bass_kernel_spmd(nc, in_maps, core_ids=list(range(N_CORES)))
    if _results_hook is not None:
        _results_hook(res)
    out = np.zeros((2, S, D), np.float32)
    for core in range(N_CORES):
        out[core // 4] += np.asarray(res.results[core]["out"], np.float32)
    return out


if __name__ == "__main__":
    nc = get_nc()
    print("built ok; instructions:",
          sum(len(bb.instructions) for bb in nc.main_func.blocks))
    from concourse.timeline_sim import TimelineSim
    print("timeline:", TimelineSim(nc).simulate())


# revision 31
# speedup vs baseline: 1.0431x; 1.0431x over previous
"""Causal multi-head attention (B=2, S=2048, D=1024, H=16) on 8 trn2
NeuronCores.

Sharding (per the head-parallel hint): core c handles batch c//4 and heads
4*(c%4) .. 4*(c%4)+3 (a 256-wide slice of the q/k/v feature dim).  W_proj is
tensor-parallel split along the head dim, so each core emits a full-shape
[S, D] partial projection output; the host sums the 4 partials per batch.

v3 strategy (all-bf16 data path, fp32 PSUM accumulation):
  - host feeds x[b].T in bf16 so the contraction dim (d) lands on partitions
  - q/k pair-0 projection runs per-contraction-chunk into 8 persistent PSUM
    banks so the PE starts on the first x chunk; input DMAs are split and
    ordered by first use
  - qT/kT in transposed [dh, s] bf16 layout (2 head-pairs of 128); v in
    natural [s, dh] layout interleaved per head as [64 v | 64 ones] so the
    AV matmul emits replicated softmax denominators
  - the attention inner loop is software-pipelined: AV(j-1) is emitted after
    scores(j), with "filler" matmuls (pair-1 q/k projection during pair-0
    attention, output projection of the previous quarter during pair-1
    attention) interleaved between strips so the PE never idles while the
    scalar engine runs Exp -- attention is Exp-bound on ACT otherwise
  - diagonal blocks masked by accumulating a bf16 matmul (strict-upper -1000
    against identity) before Exp; no max-subtraction needed
"""

import os

import numpy as np

# cache compiled executables (incl. the wrapped NEFF) across processes
os.environ.setdefault("JAX_COMPILATION_CACHE_DIR", "/tmp/jax_comp_cache")
os.environ.setdefault("JAX_PERSISTENT_CACHE_MIN_ENTRY_SIZE_BYTES", "0")
os.environ.setdefault("JAX_PERSISTENT_CACHE_MIN_COMPILE_TIME_SECS", "0")

S = 2048
D = 1024
DH = 64
P = 128
NT = S // P   # 16 sequence tiles
DC = D // P   # 8 contraction chunks
MASK_C = 1000.0
N_CORES = 8

_CACHE = {}


def _build_bass():
    from collections import deque

    import concourse.bass as bass
    import concourse.tile as tile
    from concourse import mybir

    f32 = mybir.dt.float32
    bf16 = mybir.dt.bfloat16
    EXP = mybir.ActivationFunctionType.Exp

    nc = bass.Bass("TRN2")

    xT_d = nc.dram_tensor("xT", [D, S], bf16, kind="ExternalInput")
    wq_d = nc.dram_tensor("wq_t", [D, 256], bf16, kind="ExternalInput")
    wk_d = nc.dram_tensor("wk_t", [D, 256], bf16, kind="ExternalInput")
    wv_d = nc.dram_tensor("wv_t", [D, 256], bf16, kind="ExternalInput")
    wp_d = nc.dram_tensor("wp_t", [256, D], bf16, kind="ExternalInput")
    mask_d = nc.dram_tensor("mask_lhsT", [P, P], bf16, kind="ExternalInput")
    ident_d = nc.dram_tensor("ident", [P, P], bf16, kind="ExternalInput")
    out_d = nc.dram_tensor("out", [S, D], bf16, kind="ExternalOutput")

    with tile.TileContext(nc) as tc:
        with tc.tile_pool(name="persist", bufs=1) as persist:
            qT = [persist.tile([P, S], bf16, name=f"qT{p}", tag=f"qT{p}")
                  for p in range(2)]
            kT = [persist.tile([P, S], bf16, name=f"kT{p}", tag=f"kT{p}")
                  for p in range(2)]
            v4e = [persist.tile([P, 4, P], bf16, name=f"v4e{t}", tag=f"v4e{t}")
                   for t in range(NT)]
            wp_sb = persist.tile([P, 2, D], bf16, name="wp", tag="wp")
            attnT = [persist.tile([P, S], bf16, name=f"attnT{p}",
                                  tag=f"attnT{p}") for p in range(2)]
            mask_sb = persist.tile([P, P], bf16, name="mask_sb", tag="mask_sb")
            ident_sb = persist.tile([P, P], bf16, name="ident_sb",
                                    tag="ident_sb")

            with tc.tile_pool(name="xw", bufs=1) as xw:
                xsb = xw.tile([P, DC, S], bf16, name="xsb", tag="xsb")
                wq_sb = xw.tile([P, DC, 256], bf16, name="wq", tag="wq")
                wk_sb = xw.tile([P, DC, 256], bf16, name="wk", tag="wk")
                wv_sb = xw.tile([P, DC, 256], bf16, name="wv", tag="wv")

                # DMA order = first-use order on the PE (split for latency)
                wqv = wq_d[:].rearrange("(c p) n -> p c n", p=P)
                wkv = wk_d[:].rearrange("(c p) n -> p c n", p=P)
                xv = xT_d[:].rearrange("(c p) s -> p c s", p=P)
                nc.sync.dma_start(out=wq_sb[:, 0:2, :], in_=wqv[:, 0:2, :])
                nc.sync.dma_start(out=xsb[:, 0, 0:1024], in_=xv[:, 0, 0:1024])
                nc.sync.dma_start(out=xsb[:, 0, 1024:S], in_=xv[:, 0, 1024:S])
                nc.sync.dma_start(out=xsb[:, 1:2, :], in_=xv[:, 1:2, :])
                nc.sync.dma_start(out=wq_sb[:, 2:8, :], in_=wqv[:, 2:8, :])
                for i in range(1, 4):
                    nc.sync.dma_start(
                        out=xsb[:, 2 * i:2 * i + 2, :],
                        in_=xv[:, 2 * i:2 * i + 2, :])
                nc.sync.dma_start(out=wk_sb[:], in_=wkv[:])
                nc.sync.dma_start(
                    out=wv_sb[:],
                    in_=wv_d[:].rearrange("(c p) n -> p c n", p=P))
                nc.sync.dma_start(out=mask_sb[:], in_=mask_d[:])
                nc.sync.dma_start(out=ident_sb[:], in_=ident_d[:])
                nc.sync.dma_start(
                    out=wp_sb[:],
                    in_=wp_d[:].rearrange("(c p) n -> p c n", p=P))

                # ones halves of v4e (constant)
                for t in range(NT):
                    nc.vector.memset(v4e[t][:, :, 64:P], 1.0)

                # ---- phase 1a: q for BOTH pairs, chunk-major into all 8
                # PSUM banks (1.7us of PE work per 1.46us x-chunk transfer,
                # so the PE tracks the DMA stream without starving) ----
                with tc.tile_pool(name="pk0", bufs=1, space="PSUM") as pk0:
                    qg = [pk0.tile([P, 512], f32, name=f"qg{p}{n}",
                                   tag=f"qg{p}{n}")
                          for p in range(2) for n in range(4)]
                    for c in range(DC):
                        for g, (p, n) in enumerate(
                                (p, n) for p in range(2) for n in range(4)):
                            nc.tensor.matmul(
                                qg[g][:],
                                lhsT=wq_sb[:, c, p * P:(p + 1) * P],
                                rhs=xsb[:, c, n * 512:(n + 1) * 512],
                                start=(c == 0), stop=(c == DC - 1),
                            )
                            if c == DC - 1:
                                nc.vector.tensor_copy(
                                    qT[p][:, n * 512:(n + 1) * 512], qg[g][:])

                    # ---- phase 1b: k pair 0 tile-major (reusing the q
                    # banks, per-tile deps on single copies), then v tiles
                    # 0..3 eagerly (needed by the first attention quarter)
                    for n in range(4):
                        psh = qg[n][:]
                        for c in range(DC):
                            nc.tensor.matmul(
                                psh,
                                lhsT=wk_sb[:, c, 0:P],
                                rhs=xsb[:, c, n * 512:(n + 1) * 512],
                                start=(c == 0), stop=(c == DC - 1),
                            )
                        nc.vector.tensor_copy(
                            kT[0][:, n * 512:(n + 1) * 512], psh)
                    for t in range(4):
                        psv = qg[4 + t][:, 0:256]
                        for c in range(DC):
                            nc.tensor.matmul(
                                psv,
                                lhsT=xsb[:, c, t * P:(t + 1) * P],
                                rhs=wv_sb[:, c, :],
                                start=(c == 0), stop=(c == DC - 1),
                            )
                        nc.vector.tensor_copy(
                            v4e[t][:, :, 0:64],
                            psv.rearrange("p (h d) -> p h d", h=4))

                # ------- phase 2+3: attention with interleaved fillers -------
                with tc.tile_pool(name="att", bufs=4) as att, \
                     tc.tile_pool(name="rc", bufs=4) as rc, \
                     tc.tile_pool(name="po", bufs=3) as po, \
                     tc.tile_pool(name="ps_s", bufs=2, space="PSUM") as ps_s, \
                     tc.tile_pool(name="ps_a", bufs=2, space="PSUM") as ps_a, \
                     tc.tile_pool(name="fil", bufs=2, space="PSUM") as fil:

                    v_queue = deque()
                    k1_queue = deque()
                    proj_queue = deque()
                    v_done = [4]   # tiles 0..3 built eagerly above
                    k1_done = [0]

                    def v_units():
                        # v tiles 4..15 as single-matmul filler units
                        for t in range(4, NT):
                            box = {}

                            def mkv(c, t=t, box=box):
                                def f():
                                    if "t" not in box:
                                        box["t"] = fil.tile(
                                            [P, 512], f32,
                                            name=f"vf{t}", tag="fil")
                                    nc.tensor.matmul(
                                        box["t"][:, 0:256],
                                        lhsT=xsb[:, c, t * P:(t + 1) * P],
                                        rhs=wv_sb[:, c, :],
                                        start=(c == 0), stop=(c == DC - 1),
                                    )
                                    if c == DC - 1:
                                        nc.vector.tensor_copy(
                                            v4e[t][:, :, 0:64],
                                            box["t"][:, 0:256].rearrange(
                                                "p (h d) -> p h d", h=4))
                                        v_done[0] += 1
                                return f

                            for c in range(DC):
                                yield mkv(c)

                    def k1_units():
                        # k pair-1 projection as filler units, n-major so
                        # pair-1 attention quarter qc needs groups n <= qc
                        for n in range(4):
                            box = {}

                            def mkk(c, n=n, box=box):
                                def f():
                                    if "t" not in box:
                                        box["t"] = fil.tile(
                                            [P, 512], f32,
                                            name=f"k1_{n}", tag="fil")
                                    nc.tensor.matmul(
                                        box["t"][:],
                                        lhsT=wk_sb[:, c, P:2 * P],
                                        rhs=xsb[:, c,
                                                n * 512:(n + 1) * 512],
                                        start=(c == 0), stop=(c == DC - 1),
                                    )
                                    if c == DC - 1:
                                        nc.vector.tensor_copy(
                                            kT[1][:, n * 512:(n + 1) * 512],
                                            box["t"][:])
                                        k1_done[0] += 1
                                return f

                            for c in range(DC):
                                yield mkk(c)

                    def proj_units(qc):
                        # output projection of quarter qc as filler units
                        for ti in range(4):
                            t = 4 * qc + ti
                            box = {}

                            def mkp(oc, p, t=t, ti=ti, box=box):
                                def f():
                                    if oc == 0 and p == 0:
                                        box["ot"] = po.tile(
                                            [P, D], bf16, name="ot", tag="ot")
                                    if p == 0:
                                        if qc == 3 and (ti + oc) % 2 == 1:
                                            # last quarter: widen the PSUM
                                            # rotation with freed strip tiles
                                            st = ps_s.tile(
                                                [P, 1024], f32, name="psop",
                                                tag="strip")
                                            box["ps"] = st[:, 0:512]
                                        else:
                                            box["ps"] = fil.tile(
                                                [P, 512], f32, name="pso",
                                                tag="fil")[:]
                                    nc.tensor.matmul(
                                        box["ps"],
                                        lhsT=attnT[p][:, t * P:(t + 1) * P],
                                        rhs=wp_sb[:, p,
                                                  oc * 512:(oc + 1) * 512],
                                        start=(p == 0), stop=(p == 1),
                                    )
                                    if p == 1:
                                        if qc == 3 and (ti + oc) % 2 == 0:
                                            nc.scalar.copy(
                                                box["ot"][:, oc * 512:
                                                          (oc + 1) * 512],
                                                box["ps"])
                                        else:
                                            nc.vector.tensor_copy(
                                                box["ot"][:, oc * 512:
                                                          (oc + 1) * 512],
                                                box["ps"])
                                    if p == 1 and oc == 1:
                                        nc.sync.dma_start(
                                            out=out_d[t * P:(t + 1) * P, :],
                                            in_=box["ot"][:])
                                return f

                            for oc in range(2):
                                for p in range(2):
                                    yield mkp(oc, p)

                    def take(n, queues):
                        for _ in range(n):
                            for q in queues:
                                if q:
                                    q.popleft()()
                                    break
                            else:
                                return

                    def attention(pr, qc, per_strip, queues=(),
                                  tail_quiet=False, defer_norm=False):
                        qh = [qT[pr][h * 64:(h + 1) * 64, :] for h in range(2)]
                        kh = [kT[pr][h * 64:(h + 1) * 64, :] for h in range(2)]
                        c0 = qc * 512
                        pa = [ps_a.tile([P, 512], f32, name=f"pa{pr}{qc}{h}",
                                        tag=f"pa{h}", bufs=1)
                              for h in range(2)]
                        jmax = 4 * qc + 3
                        pend = None  # deferred AV: (j, lo, w, et)
                        for j in range(jmax + 1):
                            w0 = j * P
                            lo = max(w0, c0)
                            w = c0 + 512 - lo
                            strip = ps_s.tile([P, 1024], f32,
                                              name="strip", tag="strip")
                            for h in range(2):
                                nc.tensor.matmul(
                                    strip[:, h * 512 + lo - c0:
                                          h * 512 + lo - c0 + w],
                                    lhsT=kh[h][:, w0:w0 + P],
                                    rhs=qh[h][:, lo:lo + w],
                                    start=True, stop=(j // 4 != qc),
                                    skip_group_check=True,
                                )
                            if j // 4 == qc:
                                for h in range(2):
                                    nc.tensor.matmul(
                                        strip[:, h * 512 + w0 - c0:
                                              h * 512 + w0 - c0 + P],
                                        lhsT=mask_sb[:],
                                        rhs=ident_sb[:],
                                        start=False, stop=True,
                                        skip_group_check=True,
                                    )
                            et = att.tile([P, 1024], bf16, name="et", tag="et")
                            sv = strip.rearrange("p (h q) -> p h q", h=2)
                            ev = et.rearrange("p (h q) -> p h q", h=2)
                            nc.scalar.activation(
                                out=ev[:, :, lo - c0:lo - c0 + w],
                                in_=sv[:, :, lo - c0:lo - c0 + w],
                                func=EXP)
                            if not (tail_quiet and j >= jmax - 1):
                                take(per_strip, queues)
                            if pend is not None:
                                _av(pr, pa, c0, jmax, *pend)
                            pend = (j, lo, w, et)
                        _av(pr, pa, c0, jmax, *pend,
                            defer_norm=defer_norm)
                        return pa

                    def _av(pr, pa, c0, jmax, j, lo, w, et,
                            defer_norm=False):
                        for h in range(2):
                            nc.tensor.matmul(
                                pa[h][:, lo - c0:lo - c0 + w],
                                lhsT=v4e[j][:, 2 * pr + h, :],
                                rhs=et[:, h * 512 + lo - c0:
                                       h * 512 + lo - c0 + w],
                                start=(j == 0), stop=(j == jmax),
                                skip_group_check=True,
                            )
                            if j == jmax and not defer_norm:
                                # normalize this head immediately: the DVE
                                # chain overlaps the other head's AV
                                recip = rc.tile([64, 512], f32, name="recip",
                                                tag="recip")
                                nc.vector.reciprocal(recip[:], pa[h][64:P, :])
                                nc.vector.tensor_mul(
                                    attnT[pr][h * 64:(h + 1) * 64,
                                              c0:c0 + 512],
                                    pa[h][0:64, :],
                                    recip[:],
                                )

                    v_queue.extend(v_units())
                    k1_queue.extend(k1_units())
                    for qc in range(4):
                        # quarter qc's AV consumes v tiles up to 4*qc+3
                        while v_done[0] < min(4 * qc + 4, NT) and v_queue:
                            v_queue.popleft()()
                        attention(0, qc, per_strip=2, queues=(v_queue,))
                    for qc in range(4):
                        # pair-1 quarter qc reads kT[1] cols < 512*(qc+1)
                        while v_queue:
                            v_queue.popleft()()
                        while k1_done[0] < qc + 1 and k1_queue:
                            k1_queue.popleft()()
                        pa3 = attention(1, qc, per_strip=2,
                                        queues=(k1_queue, proj_queue),
                                        tail_quiet=(qc == 3),
                                        defer_norm=(qc == 3))
                        if qc < 3:
                            proj_queue.extend(proj_units(qc))
                    while proj_queue:
                        proj_queue.popleft()()

                    # ---- tail: last quarter's normalization and projection
                    # software-pipelined per 128-query block: DVE recip/mul
                    # for block b runs beside the next tile's p0 matmuls on
                    # the PE and this tile's copies on ACT ----
                    def p0_mms(ti):
                        t = 12 + ti
                        grps = []
                        for oc in range(2):
                            pool, shape = ((fil, [P, 512]) if oc == 0
                                           else (ps_s, [P, 1024]))
                            tag = "fil" if oc == 0 else "strip"
                            ps = pool.tile(shape, f32, name=f"pt{ti}{oc}",
                                           tag=tag)[:, 0:512]
                            nc.tensor.matmul(
                                ps,
                                lhsT=attnT[0][:, t * P:(t + 1) * P],
                                rhs=wp_sb[:, 0, oc * 512:(oc + 1) * 512],
                                start=True, stop=False,
                            )
                            grps.append(ps)
                        return grps

                    grps = p0_mms(0)
                    for ti in range(4):
                        t = 12 + ti
                        c0 = 1536 + ti * P
                        for h in range(2):
                            recip = rc.tile([64, P], f32, name="recipb",
                                            tag="recipb")
                            nc.vector.reciprocal(
                                recip[:], pa3[h][64:P, ti * P:(ti + 1) * P])
                            nc.vector.tensor_mul(
                                attnT[1][h * 64:(h + 1) * 64, c0:c0 + P],
                                pa3[h][0:64, ti * P:(ti + 1) * P],
                                recip[:],
                            )
                        nxt = p0_mms(ti + 1) if ti < 3 else None
                        ot = po.tile([P, D], bf16, name="ot3", tag="ot")
                        for oc in range(2):
                            nc.tensor.matmul(
                                grps[oc],
                                lhsT=attnT[1][:, t * P:(t + 1) * P],
                                rhs=wp_sb[:, 1, oc * 512:(oc + 1) * 512],
                                start=False, stop=True,
                            )
                            if ti >= 2 and oc == 1:
                                # DVE is free once its norm blocks drain
                                nc.vector.tensor_copy(
                                    ot[:, oc * 512:(oc + 1) * 512], grps[oc])
                            else:
                                nc.scalar.copy(
                                    ot[:, oc * 512:(oc + 1) * 512], grps[oc])
                            if ti == 3:
                                nc.scalar.dma_start(
                                    out=out_d[t * P:(t + 1) * P,
                                              oc * 512:(oc + 1) * 512],
                                    in_=ot[:, oc * 512:(oc + 1) * 512])
                        if ti < 3:
                            nc.sync.dma_start(
                                out=out_d[t * P:(t + 1) * P, :], in_=ot[:])
                        grps = nxt

    return nc


def _fix_matmul_waits(nc):
    """The TRN2 ISA events struct holds exactly ONE sync-wait per
    instruction and walrus codegen refuses instructions carrying more
    ("Too many sync wait commands").  Tile emits multi-wait instructions,
    so legalize: hoist excess waits onto single-wait NoOps inserted right
    before the instruction on the same engine -- engine FIFO order
    preserves the synchronization semantics."""
    import bass_rust
    import concourse.mybir as mybir

    n = 0
    for bb in nc.main_func.blocks:
        insts = bb.instructions
        i = 0
        while i < len(insts):
            ins = insts[i]
            si = getattr(ins, "sync_info", None)
            if si is not None and len(si.on_wait) >= 2:
                for w in si.on_wait[:-1]:
                    nop = mybir.InstNoOp(name=f"I-xwait-{n}", ins=[], outs=[])
                    nop.engine = ins.engine
                    nop.sync_info = bass_rust.SyncInfo(
                        on_wait=[w], on_update=[])
                    insts.insert(i, nop)
                    n += 1
                    i += 1
                ins.sync_info = bass_rust.SyncInfo(
                    on_wait=[si.on_wait[-1]], on_update=si.on_update)
            i += 1
    return n


def get_nc(legalize=True):
    key = ("nc", legalize)
    if key not in _CACHE:
        nc = _build_bass()
        if legalize:
            _fix_matmul_waits(nc)
        _CACHE[key] = nc
    return _CACHE[key]


def make_in_maps(x, W_q, W_k, W_v, W_proj):
    import ml_dtypes

    bf = ml_dtypes.bfloat16
    x = np.asarray(x, np.float32)
    W_q = np.asarray(W_q, np.float32)
    W_k = np.asarray(W_k, np.float32)
    W_v = np.asarray(W_v, np.float32)
    W_proj = np.asarray(W_proj, np.float32)

    mask = np.triu(np.full((P, P), -MASK_C, np.float32), k=1).astype(bf)
    ident = np.eye(P, dtype=bf)

    xTs = [np.ascontiguousarray(x[b].T).astype(bf) for b in range(2)]
    in_maps = []
    for core in range(N_CORES):
        b = core // 4
        g = core % 4
        rs = slice(g * 256, (g + 1) * 256)
        in_maps.append({
            "xT": xTs[b],
            "wq_t": (np.ascontiguousarray(W_q[rs].T) / 8.0).astype(bf),
            "wk_t": np.ascontiguousarray(W_k[rs].T).astype(bf),
            "wv_t": np.ascontiguousarray(W_v[rs].T).astype(bf),
            "wp_t": np.ascontiguousarray(W_proj[:, rs].T).astype(bf),
            "mask_lhsT": mask,
            "ident": ident,
        })
    return in_maps


def kernel(x, W_q, W_k, W_v, W_proj, _results_hook=None):
    from concourse.bass_utils import run_bass_kernel_spmd

    nc = get_nc()
    in_maps = make_in_maps(x, W_q, W_k, W_v, W_proj)
    res = run_bass_kernel_spmd(nc, in_maps, core_ids=list(range(N_CORES)))
    if _results_hook is not None:
        _results_hook(res)
    out = np.zeros((2, S, D), np.float32)
    for core in range(N_CORES):
        out[core // 4] += np.asarray(res.results[core]["out"], np.float32)
    return out


if __name__ == "__main__":
    nc = get_nc()
    print("built ok; instructions:",
          sum(len(bb.instructions) for bb in nc.main_func.blocks))
    from concourse.timeline_sim import TimelineSim
    print("timeline:", TimelineSim(nc).simulate())


# revision 36
# speedup vs baseline: 1.0787x; 1.0341x over previous
"""Causal multi-head attention (B=2, S=2048, D=1024, H=16) on 8 trn2
NeuronCores.

Sharding (per the head-parallel hint): core c handles batch c//4 and heads
4*(c%4) .. 4*(c%4)+3 (a 256-wide slice of the q/k/v feature dim).  W_proj is
tensor-parallel split along the head dim, so each core emits a full-shape
[S, D] partial projection output; the host sums the 4 partials per batch.

v3 strategy (all-bf16 data path, fp32 PSUM accumulation):
  - host feeds x[b].T in bf16 so the contraction dim (d) lands on partitions
  - q/k pair-0 projection runs per-contraction-chunk into 8 persistent PSUM
    banks so the PE starts on the first x chunk; input DMAs are split and
    ordered by first use
  - qT/kT in transposed [dh, s] bf16 layout (2 head-pairs of 128); v in
    natural [s, dh] layout interleaved per head as [64 v | 64 ones] so the
    AV matmul emits replicated softmax denominators
  - the attention inner loop is software-pipelined: AV(j-1) is emitted after
    scores(j), with "filler" matmuls (pair-1 q/k projection during pair-0
    attention, output projection of the previous quarter during pair-1
    attention) interleaved between strips so the PE never idles while the
    scalar engine runs Exp -- attention is Exp-bound on ACT otherwise
  - diagonal blocks masked by accumulating a bf16 matmul (strict-upper -1000
    against identity) before Exp; no max-subtraction needed
"""

import os

import numpy as np

# cache compiled executables (incl. the wrapped NEFF) across processes
os.environ.setdefault("JAX_COMPILATION_CACHE_DIR", "/tmp/jax_comp_cache")
os.environ.setdefault("JAX_PERSISTENT_CACHE_MIN_ENTRY_SIZE_BYTES", "0")
os.environ.setdefault("JAX_PERSISTENT_CACHE_MIN_COMPILE_TIME_SECS", "0")

S = 2048
D = 1024
DH = 64
P = 128
NT = S // P   # 16 sequence tiles
DC = D // P   # 8 contraction chunks
MASK_C = 1000.0
N_CORES = 8

_CACHE = {}


def _build_bass():
    from collections import deque

    import concourse.bass as bass
    import concourse.tile as tile
    from concourse import mybir

    f32 = mybir.dt.float32
    bf16 = mybir.dt.bfloat16
    EXP = mybir.ActivationFunctionType.Exp

    nc = bass.Bass("TRN2")

    xT_d = nc.dram_tensor("xT", [D, S], bf16, kind="ExternalInput")
    wq_d = nc.dram_tensor("wq_t", [D, 256], bf16, kind="ExternalInput")
    wk_d = nc.dram_tensor("wk_t", [D, 256], bf16, kind="ExternalInput")
    wv_d = nc.dram_tensor("wv_t", [D, 256], bf16, kind="ExternalInput")
    wp_d = nc.dram_tensor("wp_t", [256, D], bf16, kind="ExternalInput")
    zmask_d = nc.dram_tensor("zmask", [P, 2 * P], bf16, kind="ExternalInput")
    out_d = nc.dram_tensor("out", [S, D], bf16, kind="ExternalOutput")

    with tile.TileContext(nc) as tc:
        with tc.tile_pool(name="persist", bufs=1) as persist:
            qT = [persist.tile([P, S], bf16, name=f"qT{p}", tag=f"qT{p}")
                  for p in range(2)]
            kT = [persist.tile([P, S], bf16, name=f"kT{p}", tag=f"kT{p}")
                  for p in range(2)]
            v4e = [persist.tile([P, 4, P], bf16, name=f"v4e{t}", tag=f"v4e{t}")
                   for t in range(NT)]
            wp_sb = persist.tile([P, 2, D], bf16, name="wp", tag="wp")
            attnT = [persist.tile([P, S], bf16, name=f"attnT{p}",
                                  tag=f"attnT{p}") for p in range(2)]
            zmask_sb = persist.tile([P, 2, P], bf16, name="zmask_sb",
                                    tag="zmask_sb")

            with tc.tile_pool(name="xw", bufs=1) as xw:
                xsb = xw.tile([P, DC, S], bf16, name="xsb", tag="xsb")
                wq_sb = xw.tile([P, DC, 256], bf16, name="wq", tag="wq")
                wk_sb = xw.tile([P, DC, 256], bf16, name="wk", tag="wk")
                wv_sb = xw.tile([P, DC, 256], bf16, name="wv", tag="wv")

                # DMA order = first-use order on the PE (split for latency)
                wqv = wq_d[:].rearrange("(c p) n -> p c n", p=P)
                wkv = wk_d[:].rearrange("(c p) n -> p c n", p=P)
                xv = xT_d[:].rearrange("(c p) s -> p c s", p=P)
                nc.sync.dma_start(out=wq_sb[:, 0:2, :], in_=wqv[:, 0:2, :])
                nc.sync.dma_start(out=xsb[:, 0, 0:1024], in_=xv[:, 0, 0:1024])
                nc.sync.dma_start(out=xsb[:, 0, 1024:S], in_=xv[:, 0, 1024:S])
                nc.sync.dma_start(out=xsb[:, 1:2, :], in_=xv[:, 1:2, :])
                nc.sync.dma_start(out=wq_sb[:, 2:8, :], in_=wqv[:, 2:8, :])
                for c in (2, 3):
                    nc.sync.dma_start(out=xsb[:, c, 0:1024],
                                      in_=xv[:, c, 0:1024])
                    nc.sync.dma_start(out=xsb[:, c, 1024:S],
                                      in_=xv[:, c, 1024:S])
                for i in (2, 3):
                    nc.sync.dma_start(
                        out=xsb[:, 2 * i:2 * i + 2, :],
                        in_=xv[:, 2 * i:2 * i + 2, :])
                nc.sync.dma_start(out=wk_sb[:], in_=wkv[:])
                nc.sync.dma_start(
                    out=wv_sb[:],
                    in_=wv_d[:].rearrange("(c p) n -> p c n", p=P))
                nc.sync.dma_start(
                    out=zmask_sb[:],
                    in_=zmask_d[:].rearrange("p (h q) -> p h q", h=2))
                nc.sync.dma_start(
                    out=wp_sb[:],
                    in_=wp_d[:].rearrange("(c p) n -> p c n", p=P))

                # ones halves of v4e (constant)
                for t in range(NT):
                    nc.vector.memset(v4e[t][:, :, 64:P], 1.0)

                # ---- phase 1a: q for BOTH pairs, chunk-major into all 8
                # PSUM banks (1.7us of PE work per 1.46us x-chunk transfer,
                # so the PE tracks the DMA stream without starving) ----
                with tc.tile_pool(name="pk0", bufs=1, space="PSUM") as pk0:
                    qg = [pk0.tile([P, 512], f32, name=f"qg{p}{n}",
                                   tag=f"qg{p}{n}")
                          for p in range(2) for n in range(4)]
                    for c in range(DC):
                        for g, (p, n) in enumerate(
                                (p, n) for p in range(2) for n in range(4)):
                            nc.tensor.matmul(
                                qg[g][:],
                                lhsT=wq_sb[:, c, p * P:(p + 1) * P],
                                rhs=xsb[:, c, n * 512:(n + 1) * 512],
                                start=(c == 0), stop=(c == DC - 1),
                            )
                            if c == DC - 1:
                                nc.vector.tensor_copy(
                                    qT[p][:, n * 512:(n + 1) * 512], qg[g][:])

                    # ---- phase 1b: k pair 0 tile-major (reusing the q
                    # banks, per-tile deps on single copies), then v tiles
                    # 0..3 eagerly (needed by the first attention quarter)
                    for n in range(4):
                        psh = qg[n][:]
                        for c in range(DC):
                            nc.tensor.matmul(
                                psh,
                                lhsT=wk_sb[:, c, 0:P],
                                rhs=xsb[:, c, n * 512:(n + 1) * 512],
                                start=(c == 0), stop=(c == DC - 1),
                            )
                        nc.vector.tensor_copy(
                            kT[0][:, n * 512:(n + 1) * 512], psh)
                    for t in range(4):
                        psv = qg[4 + t][:, 0:256]
                        for c in range(DC):
                            nc.tensor.matmul(
                                psv,
                                lhsT=xsb[:, c, t * P:(t + 1) * P],
                                rhs=wv_sb[:, c, :],
                                start=(c == 0), stop=(c == DC - 1),
                            )
                        nc.vector.tensor_copy(
                            v4e[t][:, :, 0:64],
                            psv.rearrange("p (h d) -> p h d", h=4))

                # ------- phase 2+3: attention with interleaved fillers -------
                with tc.tile_pool(name="att", bufs=6) as att, \
                     tc.tile_pool(name="rc", bufs=4) as rc, \
                     tc.tile_pool(name="po", bufs=4) as po, \
                     tc.tile_pool(name="ps_s", bufs=2, space="PSUM") as ps_s, \
                     tc.tile_pool(name="ps_a", bufs=2, space="PSUM") as ps_a, \
                     tc.tile_pool(name="fil", bufs=2, space="PSUM") as fil:

                    v_queue = deque()
                    k1_queue = deque()
                    proj_queue = deque()
                    v_done = [4]   # tiles 0..3 built eagerly above
                    k1_done = [0]

                    def v_units():
                        # v tiles 4..15 as single-matmul filler units
                        for t in range(4, NT):
                            box = {}

                            def mkv(c, t=t, box=box):
                                def f():
                                    if "t" not in box:
                                        box["t"] = fil.tile(
                                            [P, 512], f32,
                                            name=f"vf{t}", tag="fil")
                                    nc.tensor.matmul(
                                        box["t"][:, 0:256],
                                        lhsT=xsb[:, c, t * P:(t + 1) * P],
                                        rhs=wv_sb[:, c, :],
                                        start=(c == 0), stop=(c == DC - 1),
                                    )
                                    if c == DC - 1:
                                        nc.vector.tensor_copy(
                                            v4e[t][:, :, 0:64],
                                            box["t"][:, 0:256].rearrange(
                                                "p (h d) -> p h d", h=4))
                                        v_done[0] += 1
                                return f

                            for c in range(DC):
                                yield mkv(c)

                    def k1_units():
                        # k pair-1 projection as filler units, n-major so
                        # pair-1 attention quarter qc needs groups n <= qc
                        for n in range(4):
                            box = {}

                            def mkk(c, n=n, box=box):
                                def f():
                                    if "t" not in box:
                                        box["t"] = fil.tile(
                                            [P, 512], f32,
                                            name=f"k1_{n}", tag="fil")
                                    nc.tensor.matmul(
                                        box["t"][:],
                                        lhsT=wk_sb[:, c, P:2 * P],
                                        rhs=xsb[:, c,
                                                n * 512:(n + 1) * 512],
                                        start=(c == 0), stop=(c == DC - 1),
                                    )
                                    if c == DC - 1:
                                        nc.vector.tensor_copy(
                                            kT[1][:, n * 512:(n + 1) * 512],
                                            box["t"][:])
                                        k1_done[0] += 1
                                return f

                            for c in range(DC):
                                yield mkk(c)

                    def proj_units(qc):
                        # output projection of quarter qc as filler units
                        for ti in range(4):
                            t = 4 * qc + ti
                            box = {}

                            def mkp(oc, p, t=t, ti=ti, box=box):
                                def f():
                                    if oc == 0 and p == 0:
                                        box["ot"] = po.tile(
                                            [P, D], bf16, name="ot", tag="ot")
                                    if p == 0:
                                        if qc == 3 and (ti + oc) % 2 == 1:
                                            # last quarter: widen the PSUM
                                            # rotation with freed strip tiles
                                            st = ps_s.tile(
                                                [P, 1024], f32, name="psop",
                                                tag="strip")
                                            box["ps"] = st[:, 0:512]
                                        else:
                                            box["ps"] = fil.tile(
                                                [P, 512], f32, name="pso",
                                                tag="fil")[:]
                                    nc.tensor.matmul(
                                        box["ps"],
                                        lhsT=attnT[p][:, t * P:(t + 1) * P],
                                        rhs=wp_sb[:, p,
                                                  oc * 512:(oc + 1) * 512],
                                        start=(p == 0), stop=(p == 1),
                                    )
                                    if p == 1:
                                        if qc == 3 and (ti + oc) % 2 == 0:
                                            nc.scalar.copy(
                                                box["ot"][:, oc * 512:
                                                          (oc + 1) * 512],
                                                box["ps"])
                                        else:
                                            nc.vector.tensor_copy(
                                                box["ot"][:, oc * 512:
                                                          (oc + 1) * 512],
                                                box["ps"])
                                    if p == 1 and oc == 1:
                                        nc.sync.dma_start(
                                            out=out_d[t * P:(t + 1) * P, :],
                                            in_=box["ot"][:])
                                return f

                            for oc in range(2):
                                for p in range(2):
                                    yield mkp(oc, p)

                    def take(n, queues):
                        for _ in range(n):
                            for q in queues:
                                if q:
                                    q.popleft()()
                                    break
                            else:
                                return

                    def attention(pr, qc, per_strip, queues=(),
                                  tail_quiet=False, defer_norm=False):
                        qh = [qT[pr][h * 64:(h + 1) * 64, :] for h in range(2)]
                        kh = [kT[pr][h * 64:(h + 1) * 64, :] for h in range(2)]
                        c0 = qc * 512
                        pa = [ps_a.tile([P, 512], f32, name=f"pa{pr}{qc}{h}",
                                        tag=f"pa{h}", bufs=1)
                              for h in range(2)]
                        jmax = 4 * qc + 3
                        pend = None  # deferred AV: (j, lo, w, et)
                        for j in range(jmax + 1):
                            w0 = j * P
                            lo = max(w0, c0)
                            w = c0 + 512 - lo
                            strip = ps_s.tile([P, 1024], f32,
                                              name="strip", tag="strip")
                            for h in range(2):
                                nc.tensor.matmul(
                                    strip[:, h * 512 + lo - c0:
                                          h * 512 + lo - c0 + w],
                                    lhsT=kh[h][:, w0:w0 + P],
                                    rhs=qh[h][:, lo:lo + w],
                                    start=True, stop=True,
                                    skip_group_check=True,
                                )
                            et = att.tile([P, 1024], bf16, name="et", tag="et")
                            sv = strip.rearrange("p (h q) -> p h q", h=2)
                            ev = et.rearrange("p (h q) -> p h q", h=2)
                            nc.scalar.activation(
                                out=ev[:, :, lo - c0:lo - c0 + w],
                                in_=sv[:, :, lo - c0:lo - c0 + w],
                                func=EXP)
                            if j // 4 == qc:
                                # diagonal block: zero the upper triangle of
                                # exp(scores) for both heads in one DVE mul
                                bs = w0 - c0
                                nc.vector.tensor_mul(
                                    ev[:, :, bs:bs + P],
                                    ev[:, :, bs:bs + P],
                                    zmask_sb[:])
                            if not (tail_quiet and j >= jmax - 1):
                                take(per_strip, queues)
                            if pend is not None:
                                _av(pr, pa, c0, jmax, *pend)
                            pend = (j, lo, w, et)
                        _av(pr, pa, c0, jmax, *pend,
                            defer_norm=defer_norm)
                        return pa

                    def _av(pr, pa, c0, jmax, j, lo, w, et,
                            defer_norm=False):
                        # diagonal strips: the first 128 cols wait on the DVE
                        # zeroing mul; issue the rest of the AV immediately
                        diag = (lo == j * P) and w > P
                        for h in range(2):
                            if diag:
                                nc.tensor.matmul(
                                    pa[h][:, lo - c0 + P:lo - c0 + w],
                                    lhsT=v4e[j][:, 2 * pr + h, :],
                                    rhs=et[:, h * 512 + lo - c0 + P:
                                           h * 512 + lo - c0 + w],
                                    start=(j == 0), stop=False,
                                    skip_group_check=True,
                                )
                                nc.tensor.matmul(
                                    pa[h][:, lo - c0:lo - c0 + P],
                                    lhsT=v4e[j][:, 2 * pr + h, :],
                                    rhs=et[:, h * 512 + lo - c0:
                                           h * 512 + lo - c0 + P],
                                    start=(j == 0), stop=(j == jmax),
                                    skip_group_check=True,
                                )
                            else:
                                nc.tensor.matmul(
                                    pa[h][:, lo - c0:lo - c0 + w],
                                    lhsT=v4e[j][:, 2 * pr + h, :],
                                    rhs=et[:, h * 512 + lo - c0:
                                           h * 512 + lo - c0 + w],
                                    start=(j == 0), stop=(j == jmax),
                                    skip_group_check=True,
                                )
                            if j == jmax and not defer_norm:
                                # normalize this head immediately: the DVE
                                # chain overlaps the other head's AV
                                recip = rc.tile([64, 512], f32, name="recip",
                                                tag="recip")
                                nc.vector.reciprocal(recip[:], pa[h][64:P, :])
                                nc.vector.tensor_mul(
                                    attnT[pr][h * 64:(h + 1) * 64,
                                              c0:c0 + 512],
                                    pa[h][0:64, :],
                                    recip[:],
                                )

                    v_queue.extend(v_units())
                    k1_queue.extend(k1_units())
                    for qc in range(4):
                        # quarter qc's AV consumes v tiles up to 4*qc+3
                        while v_done[0] < min(4 * qc + 4, NT) and v_queue:
                            v_queue.popleft()()
                        attention(0, qc, per_strip=2, queues=(v_queue,))
                    for qc in range(4):
                        # pair-1 quarter qc reads kT[1] cols < 512*(qc+1)
                        while v_queue:
                            v_queue.popleft()()
                        while k1_done[0] < qc + 1 and k1_queue:
                            k1_queue.popleft()()
                        pa3 = attention(1, qc, per_strip=2,
                                        queues=(k1_queue, proj_queue),
                                        tail_quiet=(qc == 3),
                                        defer_norm=(qc == 3))
                        if qc < 3:
                            proj_queue.extend(proj_units(qc))
                    while proj_queue:
                        proj_queue.popleft()()

                    # ---- tail: last quarter's normalization and projection
                    # software-pipelined per 128-query block: DVE recip/mul
                    # for block b runs beside the next tile's p0 matmuls on
                    # the PE and this tile's copies on ACT ----
                    def p0_mms(ti):
                        t = 12 + ti
                        grps = []
                        for oc in range(2):
                            pool, shape = ((fil, [P, 512]) if oc == 0
                                           else (ps_s, [P, 1024]))
                            tag = "fil" if oc == 0 else "strip"
                            ps = pool.tile(shape, f32, name=f"pt{ti}{oc}",
                                           tag=tag)[:, 0:512]
                            nc.tensor.matmul(
                                ps,
                                lhsT=attnT[0][:, t * P:(t + 1) * P],
                                rhs=wp_sb[:, 0, oc * 512:(oc + 1) * 512],
                                start=True, stop=False,
                            )
                            grps.append(ps)
                        return grps

                    grps = p0_mms(0)
                    for ti in range(4):
                        t = 12 + ti
                        c0 = 1536 + ti * P
                        for h in range(2):
                            recip = rc.tile([64, P], f32, name="recipb",
                                            tag="recipb")
                            nc.vector.reciprocal(
                                recip[:], pa3[h][64:P, ti * P:(ti + 1) * P])
                            nc.vector.tensor_mul(
                                attnT[1][h * 64:(h + 1) * 64, c0:c0 + P],
                                pa3[h][0:64, ti * P:(ti + 1) * P],
                                recip[:],
                            )
                        nxt = p0_mms(ti + 1) if ti < 3 else None
                        ot = po.tile([P, D], bf16, name="ot3", tag="ot")
                        for oc in range(2):
                            nc.tensor.matmul(
                                grps[oc],
                                lhsT=attnT[1][:, t * P:(t + 1) * P],
                                rhs=wp_sb[:, 1, oc * 512:(oc + 1) * 512],
                                start=False, stop=True,
                            )
                            if ti >= 2 and oc == 1:
                                # DVE is free once its norm blocks drain
                                nc.vector.tensor_copy(
                                    ot[:, oc * 512:(oc + 1) * 512], grps[oc])
                            else:
                                nc.scalar.copy(
                                    ot[:, oc * 512:(oc + 1) * 512], grps[oc])
                            if ti == 3:
                                nc.sync.dma_start(
                                    out=out_d[t * P:(t + 1) * P,
                                              oc * 512:(oc + 1) * 512],
                                    in_=ot[:, oc * 512:(oc + 1) * 512])
                        if ti < 3:
                            nc.sync.dma_start(
                                out=out_d[t * P:(t + 1) * P, :], in_=ot[:])
                        grps = nxt

    return nc


def _fix_matmul_waits(nc):
    """The TRN2 ISA events struct holds exactly ONE sync-wait per
    instruction and walrus codegen refuses instructions carrying more
    ("Too many sync wait commands").  Tile emits multi-wait instructions,
    so legalize: hoist excess waits onto single-wait NoOps inserted right
    before the instruction on the same engine -- engine FIFO order
    preserves the synchronization semantics."""
    import bass_rust
    import concourse.mybir as mybir

    n = 0
    for bb in nc.main_func.blocks:
        insts = bb.instructions
        i = 0
        while i < len(insts):
            ins = insts[i]
            si = getattr(ins, "sync_info", None)
            if si is not None and len(si.on_wait) >= 2:
                for w in si.on_wait[:-1]:
                    nop = mybir.InstNoOp(name=f"I-xwait-{n}", ins=[], outs=[])
                    nop.engine = ins.engine
                    nop.sync_info = bass_rust.SyncInfo(
                        on_wait=[w], on_update=[])
                    insts.insert(i, nop)
                    n += 1
                    i += 1
                ins.sync_info = bass_rust.SyncInfo(
                    on_wait=[si.on_wait[-1]], on_update=si.on_update)
            i += 1
    return n


def get_nc(legalize=True):
    key = ("nc", legalize)
    if key not in _CACHE:
        nc = _build_bass()
        if legalize:
            _fix_matmul_waits(nc)
        _CACHE[key] = nc
    return _CACHE[key]


def make_in_maps(x, W_q, W_k, W_v, W_proj):
    import ml_dtypes

    bf = ml_dtypes.bfloat16
    x = np.asarray(x, np.float32)
    W_q = np.asarray(W_q, np.float32)
    W_k = np.asarray(W_k, np.float32)
    W_v = np.asarray(W_v, np.float32)
    W_proj = np.asarray(W_proj, np.float32)

    ztri = np.triu(np.ones((P, P), np.float32)).astype(bf)
    zmask = np.concatenate([ztri, ztri], axis=1)  # [P, 2*P], per-head dup

    xTs = [np.ascontiguousarray(x[b].T).astype(bf) for b in range(2)]
    in_maps = []
    for core in range(N_CORES):
        b = core // 4
        g = core % 4
        rs = slice(g * 256, (g + 1) * 256)
        in_maps.append({
            "xT": xTs[b],
            "wq_t": (np.ascontiguousarray(W_q[rs].T) / 8.0).astype(bf),
            "wk_t": np.ascontiguousarray(W_k[rs].T).astype(bf),
            "wv_t": np.ascontiguousarray(W_v[rs].T).astype(bf),
            "wp_t": np.ascontiguousarray(W_proj[:, rs].T).astype(bf),
            "zmask": zmask,
        })
    return in_maps


def kernel(x, W_q, W_k, W_v, W_proj, _results_hook=None):
    from concourse.bass_utils import run_bass_kernel_spmd

    nc = get_nc()
    in_maps = make_in_maps(x, W_q, W_k, W_v, W_proj)
    res = run_bass_kernel_spmd(nc, in_maps, core_ids=list(range(N_CORES)))
    if _results_hook is not None:
        _results_hook(res)
    out = np.zeros((2, S, D), np.float32)
    for core in range(N_CORES):
        out[core // 4] += np.asarray(res.results[core]["out"], np.float32)
    return out


if __name__ == "__main__":
    nc = get_nc()
    print("built ok; instructions:",
          sum(len(bb.instructions) for bb in nc.main_func.blocks))
    from concourse.timeline_sim import TimelineSim
    print("timeline:", TimelineSim(nc).simulate())


# revision 43
# speedup vs baseline: 1.0801x; 1.0014x over previous
"""Causal multi-head attention (B=2, S=2048, D=1024, H=16) on 8 trn2
NeuronCores.

Sharding (per the head-parallel hint): core c handles batch c//4 and heads
4*(c%4) .. 4*(c%4)+3 (a 256-wide slice of the q/k/v feature dim).  W_proj is
tensor-parallel split along the head dim, so each core emits a full-shape
[S, D] partial projection output; the host sums the 4 partials per batch.

Layout strategy (all-bf16 data path, fp32 PSUM accumulation):
  - host feeds x[b].T in bf16 so the contraction dim (d) lands on partitions
  - phase 1 computes q for BOTH head-pairs chunk-major into all 8 PSUM banks
    (1.7us of PE work per 1.46us x-chunk DMA, so the PE tracks the input
    stream), then k pair-0 and the first v tiles tile-major, recycling banks
    with per-tile dependencies on single PSUM->SBUF copies
  - qT/kT in transposed [dh, s] bf16 layout (2 head-pairs of 128); v in
    natural [s, dh] layout interleaved per head as [64 v | 64 ones] so the
    AV matmul emits replicated softmax denominators (no cross-partition
    reduction; normalization is one reciprocal + one multiply on DVE)
  - attention is Exp-bound on the scalar engine, so the inner loop is
    software-pipelined: AV(j-1) is emitted after scores(j), and "filler"
    matmuls (remaining v tiles + k pair-1 during pair-0 attention, the
    previous quarter's output projection during pair-1 attention) are
    interleaved between strips so the PE never idles while ACT runs Exp
  - causal masking: exp of the (unmasked) diagonal block is multiplied by a
    precomputed 0/1 upper-triangle on DVE (bf16 all-SBUF, 4x mode) --
    cheaper than mask matmuls and off the PE; softmax needs no
    max-subtraction (scores ~ N(0,1), exp cannot overflow)
  - the last quarter's normalization + projection run as a per-128-query
    software pipeline (DVE recip/mul | PE p0/p1 matmuls | ACT copies) to
    shorten the serial tail into the final output DMAs
"""

import os

import numpy as np

# cache compiled executables (incl. the wrapped NEFF) across processes
os.environ.setdefault("JAX_COMPILATION_CACHE_DIR", "/tmp/jax_comp_cache")
os.environ.setdefault("JAX_PERSISTENT_CACHE_MIN_ENTRY_SIZE_BYTES", "0")
os.environ.setdefault("JAX_PERSISTENT_CACHE_MIN_COMPILE_TIME_SECS", "0")

S = 2048
D = 1024
DH = 64
P = 128
NT = S // P   # 16 sequence tiles
DC = D // P   # 8 contraction chunks
MASK_C = 1000.0
N_CORES = 8

_CACHE = {}


def _build_bass():
    from collections import deque

    import concourse.bass as bass
    import concourse.tile as tile
    from concourse import mybir

    f32 = mybir.dt.float32
    bf16 = mybir.dt.bfloat16
    EXP = mybir.ActivationFunctionType.Exp

    nc = bass.Bass("TRN2")

    xT_d = nc.dram_tensor("xT", [D, S], bf16, kind="ExternalInput")
    wq_d = nc.dram_tensor("wq_t", [D, 256], bf16, kind="ExternalInput")
    wk_d = nc.dram_tensor("wk_t", [D, 256], bf16, kind="ExternalInput")
    wv_d = nc.dram_tensor("wv_t", [D, 256], bf16, kind="ExternalInput")
    wp_d = nc.dram_tensor("wp_t", [256, D], bf16, kind="ExternalInput")
    zmask_d = nc.dram_tensor("zmask", [P, 2 * P], bf16, kind="ExternalInput")
    out_d = nc.dram_tensor("out", [S, D], bf16, kind="ExternalOutput")

    with tile.TileContext(nc) as tc:
        with tc.tile_pool(name="persist", bufs=1) as persist:
            qT = [persist.tile([P, S], bf16, name=f"qT{p}", tag=f"qT{p}")
                  for p in range(2)]
            kT = [persist.tile([P, S], bf16, name=f"kT{p}", tag=f"kT{p}")
                  for p in range(2)]
            v4e = [persist.tile([P, 4, P], bf16, name=f"v4e{t}", tag=f"v4e{t}")
                   for t in range(NT)]
            wp_sb = persist.tile([P, 2, D], bf16, name="wp", tag="wp")
            attnT = [persist.tile([P, S], bf16, name=f"attnT{p}",
                                  tag=f"attnT{p}") for p in range(2)]
            zmask_sb = persist.tile([P, 2, P], bf16, name="zmask_sb",
                                    tag="zmask_sb")

            with tc.tile_pool(name="xw", bufs=1) as xw:
                xsb = xw.tile([P, DC, S], bf16, name="xsb", tag="xsb")
                wq_sb = xw.tile([P, DC, 256], bf16, name="wq", tag="wq")
                wk_sb = xw.tile([P, DC, 256], bf16, name="wk", tag="wk")
                wv_sb = xw.tile([P, DC, 256], bf16, name="wv", tag="wv")

                # DMA order = first-use order on the PE (split for latency)
                wqv = wq_d[:].rearrange("(c p) n -> p c n", p=P)
                wkv = wk_d[:].rearrange("(c p) n -> p c n", p=P)
                xv = xT_d[:].rearrange("(c p) s -> p c s", p=P)
                nc.sync.dma_start(out=wq_sb[:, 0:2, :], in_=wqv[:, 0:2, :])
                nc.sync.dma_start(out=xsb[:, 0, 0:1024], in_=xv[:, 0, 0:1024])
                nc.sync.dma_start(out=xsb[:, 0, 1024:S], in_=xv[:, 0, 1024:S])
                nc.sync.dma_start(out=xsb[:, 1:2, :], in_=xv[:, 1:2, :])
                nc.sync.dma_start(out=wq_sb[:, 2:8, :], in_=wqv[:, 2:8, :])
                for c in (2, 3):
                    nc.sync.dma_start(out=xsb[:, c, 0:1024],
                                      in_=xv[:, c, 0:1024])
                    nc.sync.dma_start(out=xsb[:, c, 1024:S],
                                      in_=xv[:, c, 1024:S])
                for i in (2, 3):
                    nc.sync.dma_start(
                        out=xsb[:, 2 * i:2 * i + 2, :],
                        in_=xv[:, 2 * i:2 * i + 2, :])
                nc.sync.dma_start(out=wk_sb[:], in_=wkv[:])
                nc.sync.dma_start(
                    out=wv_sb[:],
                    in_=wv_d[:].rearrange("(c p) n -> p c n", p=P))
                nc.sync.dma_start(
                    out=zmask_sb[:],
                    in_=zmask_d[:].rearrange("p (h q) -> p h q", h=2))
                nc.sync.dma_start(
                    out=wp_sb[:],
                    in_=wp_d[:].rearrange("(c p) n -> p c n", p=P))

                # ones halves of v4e (constant)
                for t in range(NT):
                    nc.vector.memset(v4e[t][:, :, 64:P], 1.0)

                # ---- phase 1a: q for BOTH pairs, chunk-major into all 8
                # PSUM banks (1.7us of PE work per 1.46us x-chunk transfer,
                # so the PE tracks the DMA stream without starving) ----
                with tc.tile_pool(name="pk0", bufs=1, space="PSUM") as pk0:
                    qg = [pk0.tile([P, 512], f32, name=f"qg{p}{n}",
                                   tag=f"qg{p}{n}")
                          for p in range(2) for n in range(4)]
                    for c in range(DC):
                        for g, (p, n) in enumerate(
                                (p, n) for p in range(2) for n in range(4)):
                            nc.tensor.matmul(
                                qg[g][:],
                                lhsT=wq_sb[:, c, p * P:(p + 1) * P],
                                rhs=xsb[:, c, n * 512:(n + 1) * 512],
                                start=(c == 0), stop=(c == DC - 1),
                            )
                            if c == DC - 1:
                                nc.vector.tensor_copy(
                                    qT[p][:, n * 512:(n + 1) * 512], qg[g][:])

                    # ---- phase 1b: k pair 0 tile-major (reusing the q
                    # banks, per-tile deps on single copies), then v tiles
                    # 0..3 eagerly (needed by the first attention quarter)
                    for n in range(4):
                        psh = qg[n][:]
                        for c in range(DC):
                            nc.tensor.matmul(
                                psh,
                                lhsT=wk_sb[:, c, 0:P],
                                rhs=xsb[:, c, n * 512:(n + 1) * 512],
                                start=(c == 0), stop=(c == DC - 1),
                            )
                        nc.vector.tensor_copy(
                            kT[0][:, n * 512:(n + 1) * 512], psh)
                    for t in range(4):
                        psv = qg[4 + t][:, 0:256]
                        for c in range(DC):
                            nc.tensor.matmul(
                                psv,
                                lhsT=xsb[:, c, t * P:(t + 1) * P],
                                rhs=wv_sb[:, c, :],
                                start=(c == 0), stop=(c == DC - 1),
                            )
                        nc.vector.tensor_copy(
                            v4e[t][:, :, 0:64],
                            psv.rearrange("p (h d) -> p h d", h=4))

                # ------- phase 2+3: attention with interleaved fillers -------
                with tc.tile_pool(name="att", bufs=6) as att, \
                     tc.tile_pool(name="rc", bufs=6) as rc, \
                     tc.tile_pool(name="po", bufs=6) as po, \
                     tc.tile_pool(name="ps_s", bufs=2, space="PSUM") as ps_s, \
                     tc.tile_pool(name="ps_a", bufs=2, space="PSUM") as ps_a, \
                     tc.tile_pool(name="fil", bufs=2, space="PSUM") as fil:

                    v_queue = deque()
                    k1_queue = deque()
                    proj_queue = deque()
                    v_done = [4]   # tiles 0..3 built eagerly above
                    k1_done = [0]

                    def v_units():
                        # v tiles 4..15 as single-matmul filler units
                        for t in range(4, NT):
                            box = {}

                            def mkv(c, t=t, box=box):
                                def f():
                                    if "t" not in box:
                                        box["t"] = fil.tile(
                                            [P, 512], f32,
                                            name=f"vf{t}", tag="fil")
                                    nc.tensor.matmul(
                                        box["t"][:, 0:256],
                                        lhsT=xsb[:, c, t * P:(t + 1) * P],
                                        rhs=wv_sb[:, c, :],
                                        start=(c == 0), stop=(c == DC - 1),
                                    )
                                    if c == DC - 1:
                                        nc.vector.tensor_copy(
                                            v4e[t][:, :, 0:64],
                                            box["t"][:, 0:256].rearrange(
                                                "p (h d) -> p h d", h=4))
                                        v_done[0] += 1
                                return f

                            for c in range(DC):
                                yield mkv(c)

                    def k1_units():
                        # k pair-1 projection as filler units, n-major so
                        # pair-1 attention quarter qc needs groups n <= qc
                        for n in range(4):
                            box = {}

                            def mkk(c, n=n, box=box):
                                def f():
                                    if "t" not in box:
                                        box["t"] = fil.tile(
                                            [P, 512], f32,
                                            name=f"k1_{n}", tag="fil")
                                    nc.tensor.matmul(
                                        box["t"][:],
                                        lhsT=wk_sb[:, c, P:2 * P],
                                        rhs=xsb[:, c,
                                                n * 512:(n + 1) * 512],
                                        start=(c == 0), stop=(c == DC - 1),
                                    )
                                    if c == DC - 1:
                                        nc.vector.tensor_copy(
                                            kT[1][:, n * 512:(n + 1) * 512],
                                            box["t"][:])
                                        k1_done[0] += 1
                                return f

                            for c in range(DC):
                                yield mkk(c)

                    def proj_units(qc):
                        # output projection of quarter qc as filler units
                        for ti in range(4):
                            t = 4 * qc + ti
                            box = {}

                            def mkp(oc, p, t=t, ti=ti, box=box):
                                def f():
                                    if oc == 0 and p == 0:
                                        box["ot"] = po.tile(
                                            [P, D], bf16, name="ot", tag="ot")
                                    if p == 0:
                                        if qc == 3 and (ti + oc) % 2 == 1:
                                            # last quarter: widen the PSUM
                                            # rotation with freed strip tiles
                                            st = ps_s.tile(
                                                [P, 1024], f32, name="psop",
                                                tag="strip")
                                            box["ps"] = st[:, 0:512]
                                        else:
                                            box["ps"] = fil.tile(
                                                [P, 512], f32, name="pso",
                                                tag="fil")[:]
                                    nc.tensor.matmul(
                                        box["ps"],
                                        lhsT=attnT[p][:, t * P:(t + 1) * P],
                                        rhs=wp_sb[:, p,
                                                  oc * 512:(oc + 1) * 512],
                                        start=(p == 0), stop=(p == 1),
                                    )
                                    if p == 1:
                                        if qc == 3 and (ti + oc) % 2 == 0:
                                            nc.scalar.copy(
                                                box["ot"][:, oc * 512:
                                                          (oc + 1) * 512],
                                                box["ps"])
                                        else:
                                            nc.vector.tensor_copy(
                                                box["ot"][:, oc * 512:
                                                          (oc + 1) * 512],
                                                box["ps"])
                                    if p == 1 and oc == 1:
                                        nc.sync.dma_start(
                                            out=out_d[t * P:(t + 1) * P, :],
                                            in_=box["ot"][:])
                                return f

                            for oc in range(2):
                                for p in range(2):
                                    yield mkp(oc, p)

                    def take(n, queues):
                        for _ in range(n):
                            for q in queues:
                                if q:
                                    q.popleft()()
                                    break
                            else:
                                return

                    def attention(pr, qc, per_strip, queues=(),
                                  tail_quiet=False, defer_norm=False):
                        qh = [qT[pr][h * 64:(h + 1) * 64, :] for h in range(2)]
                        kh = [kT[pr][h * 64:(h + 1) * 64, :] for h in range(2)]
                        c0 = qc * 512
                        pa = [ps_a.tile([P, 512], f32, name=f"pa{pr}{qc}{h}",
                                        tag=f"pa{h}", bufs=1)
                              for h in range(2)]
                        jmax = 4 * qc + 3
                        pend = None  # deferred AV: (j, lo, w, et)
                        for j in range(jmax + 1):
                            w0 = j * P
                            lo = max(w0, c0)
                            w = c0 + 512 - lo
                            strip = ps_s.tile([P, 1024], f32,
                                              name="strip", tag="strip")
                            for h in range(2):
                                nc.tensor.matmul(
                                    strip[:, h * 512 + lo - c0:
                                          h * 512 + lo - c0 + w],
                                    lhsT=kh[h][:, w0:w0 + P],
                                    rhs=qh[h][:, lo:lo + w],
                                    start=True, stop=True,
                                    skip_group_check=True,
                                )
                            et = att.tile([P, 1024], bf16, name="et", tag="et")
                            sv = strip.rearrange("p (h q) -> p h q", h=2)
                            ev = et.rearrange("p (h q) -> p h q", h=2)
                            nc.scalar.activation(
                                out=ev[:, :, lo - c0:lo - c0 + w],
                                in_=sv[:, :, lo - c0:lo - c0 + w],
                                func=EXP)
                            if j // 4 == qc:
                                # diagonal block: zero the upper triangle of
                                # exp(scores) for both heads in one DVE mul
                                bs = w0 - c0
                                nc.vector.tensor_mul(
                                    ev[:, :, bs:bs + P],
                                    ev[:, :, bs:bs + P],
                                    zmask_sb[:])
                            if not (tail_quiet and j >= jmax - 1):
                                take(per_strip, queues)
                            if pend is not None:
                                _av(pr, pa, c0, jmax, *pend)
                            pend = (j, lo, w, et)
                        _av(pr, pa, c0, jmax, *pend,
                            defer_norm=defer_norm)
                        return pa

                    def _av(pr, pa, c0, jmax, j, lo, w, et,
                            defer_norm=False):
                        for h in range(2):
                            nc.tensor.matmul(
                                pa[h][:, lo - c0:lo - c0 + w],
                                lhsT=v4e[j][:, 2 * pr + h, :],
                                rhs=et[:, h * 512 + lo - c0:
                                       h * 512 + lo - c0 + w],
                                start=(j == 0), stop=(j == jmax),
                                skip_group_check=True,
                            )
                            if j == jmax and not defer_norm:
                                # normalize this head immediately: the DVE
                                # chain overlaps the other head's AV
                                recip = rc.tile([64, 512], f32, name="recip",
                                                tag="recip")
                                nc.vector.reciprocal(recip[:], pa[h][64:P, :])
                                nc.vector.tensor_mul(
                                    attnT[pr][h * 64:(h + 1) * 64,
                                              c0:c0 + 512],
                                    pa[h][0:64, :],
                                    recip[:],
                                )

                    v_queue.extend(v_units())
                    k1_queue.extend(k1_units())
                    for qc in range(4):
                        # quarter qc's AV consumes v tiles up to 4*qc+3
                        while v_done[0] < min(4 * qc + 4, NT) and v_queue:
                            v_queue.popleft()()
                        attention(0, qc, per_strip=2, queues=(v_queue,))
                    for qc in range(4):
                        # pair-1 quarter qc reads kT[1] cols < 512*(qc+1)
                        while v_queue:
                            v_queue.popleft()()
                        while k1_done[0] < qc + 1 and k1_queue:
                            k1_queue.popleft()()
                        pa3 = attention(1, qc, per_strip=2,
                                        queues=(k1_queue, proj_queue),
                                        tail_quiet=(qc == 3),
                                        defer_norm=(qc == 3))
                        if qc < 3:
                            proj_queue.extend(proj_units(qc))
                    while proj_queue:
                        proj_queue.popleft()()

                    # ---- tail: last quarter's normalization and projection
                    # software-pipelined per 128-query block: DVE recip/mul
                    # for block b runs beside the next tile's p0 matmuls on
                    # the PE and this tile's copies on ACT ----
                    def p0_mms(ti):
                        t = 12 + ti
                        grps = []
                        for oc in range(2):
                            pool, shape = ((fil, [P, 512]) if oc == 0
                                           else (ps_s, [P, 1024]))
                            tag = "fil" if oc == 0 else "strip"
                            ps = pool.tile(shape, f32, name=f"pt{ti}{oc}",
                                           tag=tag)[:, 0:512]
                            nc.tensor.matmul(
                                ps,
                                lhsT=attnT[0][:, t * P:(t + 1) * P],
                                rhs=wp_sb[:, 0, oc * 512:(oc + 1) * 512],
                                start=True, stop=False,
                            )
                            grps.append(ps)
                        return grps

                    grps = p0_mms(0)
                    for ti in range(4):
                        t = 12 + ti
                        c0 = 1536 + ti * P
                        for h in range(2):
                            recip = rc.tile([64, P], f32, name="recipb",
                                            tag="recipb")
                            nc.vector.reciprocal(
                                recip[:], pa3[h][64:P, ti * P:(ti + 1) * P])
                            nc.vector.tensor_mul(
                                attnT[1][h * 64:(h + 1) * 64, c0:c0 + P],
                                pa3[h][0:64, ti * P:(ti + 1) * P],
                                recip[:],
                            )
                        nxt = p0_mms(ti + 1) if ti < 3 else None
                        ot = po.tile([P, D], bf16, name="ot3", tag="ot")
                        for oc in range(2):
                            nc.tensor.matmul(
                                grps[oc],
                                lhsT=attnT[1][:, t * P:(t + 1) * P],
                                rhs=wp_sb[:, 1, oc * 512:(oc + 1) * 512],
                                start=False, stop=True,
                            )
                            if ti >= 2 and oc == 1:
                                # DVE is free once its norm blocks drain
                                nc.vector.tensor_copy(
                                    ot[:, oc * 512:(oc + 1) * 512], grps[oc])
                            else:
                                nc.scalar.copy(
                                    ot[:, oc * 512:(oc + 1) * 512], grps[oc])
                            if ti == 3:
                                nc.sync.dma_start(
                                    out=out_d[t * P:(t + 1) * P,
                                              oc * 512:(oc + 1) * 512],
                                    in_=ot[:, oc * 512:(oc + 1) * 512])
                        if ti < 3:
                            nc.sync.dma_start(
                                out=out_d[t * P:(t + 1) * P, :], in_=ot[:])
                        grps = nxt

    return nc


def _fix_matmul_waits(nc):
    """The TRN2 ISA events struct holds exactly ONE sync-wait per
    instruction and walrus codegen refuses instructions carrying more
    ("Too many sync wait commands").  Tile emits multi-wait instructions,
    so legalize: hoist excess waits onto single-wait NoOps inserted right
    before the instruction on the same engine -- engine FIFO order
    preserves the synchronization semantics."""
    import bass_rust
    import concourse.mybir as mybir

    n = 0
    for bb in nc.main_func.blocks:
        insts = bb.instructions
        i = 0
        while i < len(insts):
            ins = insts[i]
            si = getattr(ins, "sync_info", None)
            if si is not None and len(si.on_wait) >= 2:
                for w in si.on_wait[:-1]:
                    nop = mybir.InstNoOp(name=f"I-xwait-{n}", ins=[], outs=[])
                    nop.engine = ins.engine
                    nop.sync_info = bass_rust.SyncInfo(
                        on_wait=[w], on_update=[])
                    insts.insert(i, nop)
                    n += 1
                    i += 1
                ins.sync_info = bass_rust.SyncInfo(
                    on_wait=[si.on_wait[-1]], on_update=si.on_update)
            i += 1
    return n


def get_nc(legalize=True):
    key = ("nc", legalize)
    if key not in _CACHE:
        nc = _build_bass()
        if legalize:
            _fix_matmul_waits(nc)
        _CACHE[key] = nc
    return _CACHE[key]


def make_in_maps(x, W_q, W_k, W_v, W_proj):
    import ml_dtypes

    bf = ml_dtypes.bfloat16
    x = np.asarray(x, np.float32)
    W_q = np.asarray(W_q, np.float32)
    W_k = np.asarray(W_k, np.float32)
    W_v = np.asarray(W_v, np.float32)
    W_proj = np.asarray(W_proj, np.float32)

    ztri = np.triu(np.ones((P, P), np.float32)).astype(bf)
    zmask = np.concatenate([ztri, ztri], axis=1)  # [P, 2*P], per-head dup

    xTs = [np.ascontiguousarray(x[b].T).astype(bf) for b in range(2)]
    in_maps = []
    for core in range(N_CORES):
        b = core // 4
        g = core % 4
        rs = slice(g * 256, (g + 1) * 256)
        in_maps.append({
            "xT": xTs[b],
            "wq_t": (np.ascontiguousarray(W_q[rs].T) / 8.0).astype(bf),
            "wk_t": np.ascontiguousarray(W_k[rs].T).astype(bf),
            "wv_t": np.ascontiguousarray(W_v[rs].T).astype(bf),
            "wp_t": np.ascontiguousarray(W_proj[:, rs].T).astype(bf),
            "zmask": zmask,
        })
    return in_maps


def kernel(x, W_q, W_k, W_v, W_proj, _results_hook=None):
    from concourse.bass_utils import run_bass_kernel_spmd

    nc = get_nc()
    in_maps = make_in_maps(x, W_q, W_k, W_v, W_proj)
    res = run_bass_kernel_spmd(nc, in_maps, core_ids=list(range(N_CORES)))
    if _results_hook is not None:
        _results_hook(res)
    out = np.zeros((2, S, D), np.float32)
    for core in range(N_CORES):
        out[core // 4] += np.asarray(res.results[core]["out"], np.float32)
    return out


if __name__ == "__main__":
    nc = get_nc()
    print("built ok; instructions:",
          sum(len(bb.instructions) for bb in nc.main_func.blocks))
    from concourse.timeline_sim import TimelineSim
    print("timeline:", TimelineSim(nc).simulate())
